# revision 14
# baseline (speedup 1.0000x reference)
"""Trainium2 Bass kernel for BlockFFTDirectPrior (v5).

Computes out = irfft(einsum('bjn,ijn->bin', rfft(x_blocks), conj(W)))
reshaped to [B, 4096], for x [4096, 4096] f32, W [16, 16, 129] complex
(block size 256).

Strategy: data-parallel over the batch axis across 8 NeuronCores (512 rows
each). The host pre-transposes each core's x slice to [d, b] layout and
casts to fp16 (no on-chip transpose stage, half the HBM traffic); the
kernel writes its output transposed [d, b] fp16 and the host transposes/
upcasts back.  All on-chip operands are fp16: the PE streams 2-byte moving
operands at 1 column/cycle (vs ~2 cycles for fp32/fp32r) and all
SBUF<->SBUF regroup DMAs move half the bytes.

Per core, three PE stages (no transposes):

  F: real DFT per input block j as matmuls (contract t in 2 chunks of 128)
       -> zf[n, c, j, b]; c=0 rows hold Re X[n] (n=0..127), c=1 rows hold
       Im X[n] for n>=1 and X[128] (real Nyquist bin) in the n=0 slot.
  E: per-frequency 16x16 complex mixing packed as real 2x2 blocks, four
     frequencies per 128x128 matmul -> one matmul per frequency group g4
     (32 total, no accumulation).  The Nyquist product is folded into the
     g4=0 operator (it lands in the otherwise-meaningless Im Y[0] slot).
  I: real inverse DFT with the operator stationary: out.T[m, b] chunks;
     only 4 distinct stationaries (dmat[c', mp]).

Between stages, two partition regroups (F->E "r1", E->I "r2") ride
round-robin on the three DMA rings (sync/scalar HWDGE + gpsimd SWDGE).
SBUF<->SBUF DMA costs read+write fabric-port bandwidth, so the batch is
processed as two pipelined halves of 256: each half's regroup burst
overlaps the other half's matmuls, keeping the PE HAM clock gate warm and
hiding most of the ~25 MB of DMA port traffic.  PSUM->SBUF copies pair
two matmul outputs per copy and alternate between DVE and ACT.
"""

import os
import numpy as np
from contextlib import ExitStack

import concourse.bass as bass
import concourse.tile as tile
from concourse import bacc, mybir
from concourse.bass_utils import run_bass_kernel_spmd

NCORES = 8
B_FULL, D_IN, D_OUT, BS = 4096, 4096, 4096, 256
BC = B_FULL // NCORES          # 512 batch rows per core
BH = BC // 2                   # 256 batch rows per pipelined half
KIN = KOUT = 16
F16 = mybir.dt.float16
F32 = mybir.dt.float32

_CACHE = {}
LAST_RESULTS = None            # BassKernelResults of the most recent run


def _build_consts(W_real, W_imag):
    """Constant matrices in the exact SBUF layouts the kernel reads.

    Frequency slot r in stage-F output order is frequency n = r, grouped
    for stage E as (g4 = n // 4, f_loc = n % 4).  Stage-I row order is
    n2 = f_loc * 32 + g4 (the order regroup-2's flat pairing produces).
    """
    f16 = np.float16
    t = np.arange(BS)
    n = np.arange(128)
    ang = 2.0 * np.pi / BS

    # zf physical row for frequency n (g4 = n//4, f = n%4) is
    # ROW1[n] = 16*(g4//4) + g4%4 + 4*f: each group's 4 rows sit at
    # partition stride 4 so every regroup-1 DMA's reads spread over 4
    # SDMA engines (descriptor->engine assignment keys on src partition).
    ROW1 = np.empty(128, dtype=np.int64)
    for nf_ in range(128):
        g4_, f_ = nf_ // 4, nf_ % 4
        ROW1[nf_] = 16 * (g4_ // 4) + (g4_ % 4) + 4 * f_

    # cfs[t_lo, tc, c, row]: column `row` holds frequency n with ROW1[n]=row
    cfs = np.zeros((128, 2, 2, 128), dtype=np.float32)
    C0 = np.cos(ang * np.outer(t, n))
    C1 = np.empty((BS, 128), dtype=np.float64)
    C1[:, 0] = np.cos(np.pi * t)                      # Nyquist row -> Xr[128]
    C1[:, 1:] = -np.sin(ang * np.outer(t, np.arange(1, 128)))
    inv1 = np.argsort(ROW1)                           # row -> frequency
    for tc in range(2):
        cfs[:, tc, 0, :] = C0[tc * 128:(tc + 1) * 128][:, inv1]
        cfs[:, tc, 1, :] = C1[tc * 128:(tc + 1) * 128][:, inv1]

    # yy physical row for (c', i, f): COL2 = c'*64 + 16*(i//4) + i%4 + 4*f
    # (same stride-4 trick for regroup-2's source reads)
    def COL2(cp_, i_, f_):
        return cp_ * 64 + 16 * (i_ // 4) + (i_ % 4) + 4 * f_

    # wek[row, g4, col]; row = f_loc*32 + c*16 + j, col = COL2(c', i, f_loc)
    wek = np.zeros((128, 32, 128), dtype=np.float32)
    fl = np.arange(4)[:, None, None]                  # f_loc
    ii = np.arange(KOUT)[None, :, None]
    jj = np.arange(KIN)[None, None, :]
    for g4 in range(32):
        nf = 4 * g4 + fl                              # frequency
        Wr = W_real[ii, jj, nf]
        Wi = W_imag[ii, jj, nf]
        # Yr = Wr*Xr + Wi*Xi ; Yi = Wr*Xi - Wi*Xr
        col0 = 0 * 64 + 16 * (ii // 4) + (ii % 4) + 4 * fl
        col1 = 64 + 16 * (ii // 4) + (ii % 4) + 4 * fl
        wek[fl * 32 + 0 * 16 + jj, g4, col0] = Wr
        wek[fl * 32 + 1 * 16 + jj, g4, col0] = Wi
        wek[fl * 32 + 1 * 16 + jj, g4, col1] = Wr
        wek[fl * 32 + 0 * 16 + jj, g4, col1] = -Wi
    # Nyquist fold: Im Y[0] slot carries Yr[128] = Wr[:, :, 128] * Xr[128]
    for i in range(KOUT):
        for j in range(KIN):
            wek[0 * 32 + 1 * 16 + j, 0, COL2(1, i, 0)] = W_real[i, j, 128]
            wek[0 * 32 + 0 * 16 + j, 0, COL2(1, i, 0)] = 0.0

    # dmat[n2, c', mp, m_lo]
    m = np.arange(BS)
    dmat = np.zeros((128, 2, 2, 128), dtype=np.float32)
    for nf in range(128):
        f_loc, g4 = nf % 4, nf // 4
        n2 = f_loc * 32 + g4
        if nf == 0:
            d0 = np.full(BS, 1.0 / BS)
            d1 = ((-1.0) ** m) / BS                   # Nyquist slot
        else:
            d0 = (2.0 / BS) * np.cos(ang * nf * m)
            d1 = -(2.0 / BS) * np.sin(ang * nf * m)
        for mp in range(2):
            dmat[n2, 0, mp] = d0[mp * 128:(mp + 1) * 128]
            dmat[n2, 1, mp] = d1[mp * 128:(mp + 1) * 128]

    return {
        "cfs": cfs.astype(f16),
        "wek": wek.astype(f16),
        "dmat": dmat.astype(f16),
    }


def _build_program():
    nc = bacc.Bacc(
        "TRN2", target_bir_lowering=False, debug=False, num_devices=NCORES
    )
    # xt[t_lo, s, b]: host-pretransposed fp16 x slice; d = s*128 + t_lo
    xt_d = nc.dram_tensor("xt", [128, 32, BC], F16, kind="ExternalInput").ap()
    cfs_d = nc.dram_tensor("cfs", [128, 2, 2, 128], F16, kind="ExternalInput").ap()
    wek_d = nc.dram_tensor("wek", [128, 32, 128], F16, kind="ExternalInput").ap()
    dmat_d = nc.dram_tensor("dmat", [128, 2, 2, 128], F16, kind="ExternalInput").ap()
    # out[m_lo, oc, b]: transposed fp16 output; d = oc*128 + m_lo, oc = i*2+mp
    out_d = nc.dram_tensor("out", [128, 32, BC], F16, kind="ExternalOutput").ap()

    cp_state = [0]
    ring_state = [0]

    with tile.TileContext(nc) as tc, ExitStack() as ctx:
        def copy(dst, src):
            # alternate PSUM->SBUF copies between DVE and ACT (ACT issues no
            # DMAs in this kernel, so copies never queue behind dispatches)
            if cp_state[0] % 2 == 0:
                nc.vector.tensor_copy(dst, src)
            else:
                nc.scalar.copy(dst, src)
            cp_state[0] += 1

        def ring():
            # weighted split: SWDGE (gpsimd) spreads descriptors evenly over
            # all 16 SDMA engines but dispatches serially (~0.8us); HWDGE
            # (sync) dispatches fast but skews onto 4 hot engines for these
            # few-source-partition patterns.  2:1 gpsimd:sync balances them.
            r = (nc.gpsimd, nc.gpsimd, nc.sync)[ring_state[0] % 3]
            ring_state[0] += 1
            return r

        consts = ctx.enter_context(tc.tile_pool(name="consts", bufs=1))
        xin_p = ctx.enter_context(tc.tile_pool(name="xin", bufs=1))
        zf_p = ctx.enter_context(tc.tile_pool(name="zf", bufs=1))
        yy_p = ctx.enter_context(tc.tile_pool(name="yy", bufs=1))
        yh_p = ctx.enter_context(tc.tile_pool(name="yh", bufs=1))
        gg_p = ctx.enter_context(tc.tile_pool(name="gg", bufs=1))
        os_p = ctx.enter_context(tc.tile_pool(name="os", bufs=3))
        ps = ctx.enter_context(tc.tile_pool(name="ps", bufs=4, space="PSUM"))

        cfs = consts.tile([128, 2, 2, 128], F16)
        wek = consts.tile([128, 32, 128], F16)
        dmat = consts.tile([128, 2, 2, 128], F16)
        wrm = consts.tile([128, 128], F16)

        xin = xin_p.tile([128, 32, BC], F16, tag="xin")
        # x chunks interleave sync/scalar (plain 128-partition patterns
        # spread fine on HWDGE) in stage-F consumption order, 512KB each;
        # gpsimd stays empty so regroup-1 dispatches the moment zf is ready
        nc.scalar.dma_start(cfs[:], cfs_d)
        for k in range(4):
            nc.sync.dma_start(xin[:, 8 * k:8 * k + 4, :],
                              xt_d[:, 8 * k:8 * k + 4, :])
            nc.scalar.dma_start(xin[:, 8 * k + 4:8 * k + 8, :],
                                xt_d[:, 8 * k + 4:8 * k + 8, :])
        nc.scalar.dma_start(wek[:], wek_d)
        nc.scalar.dma_start(dmat[:], dmat_d)

        # ---- PE warm-up: back-to-back matmuls on a memset tile (no DMA
        # dependency) hold the PE busy through the input-DMA latency so the
        # HAM clock gate is at 8/8 (2.4 GHz) when stage F starts.
        nc.vector.memset(wrm[:], 0)
        pw = ps.tile([128, 2, BC], F32, tag="ps")
        for _ in range(48):
            nc.tensor.matmul(pw[:, 0, 0:128], wrm[:], wrm[:],
                             start=True, stop=True)

        # ---- stage F: real DFT per block j (4 matmuls each, N=512)
        zf = zf_p.tile([128, 2, KIN, BC], F16, tag="zf")
        for j in range(KIN):
            pf = ps.tile([128, 2, BC], F32, tag="ps")
            for c in range(2):
                for tcx in range(2):
                    nc.tensor.matmul(
                        pf[:, c, :],
                        cfs[:, tcx, c, :],
                        xin[:, 2 * j + tcx, :],
                        start=(tcx == 0),
                        stop=(tcx == 1),
                    )
            copy(zf[:, :, j, :], pf[:])

        # ---- regroup1 + stage E, pipelined per group pair
        # gg[p, b] = zf[ROW1(4*g4 + f_loc), c, j, b], p = f_loc*32 + c*16 + j;
        # group g4's four zf rows sit at stride 4 (rows 16*(g4//4) + g4%4
        # + 4*f) so each DMA's source reads spread over 4 SDMA engines
        yy = yy_p.tile([128, 32, BC], F16, tag="yy")
        gg = gg_p.tile([128, 32, BC], F16, tag="gg")
        for g4 in range(32):
            base = 16 * (g4 // 4) + (g4 % 4)
            ring().dma_start(gg[:, g4, :], zf[base:base + 13:4, :, :, :])
        for g4 in range(0, 32, 2):
            pe = ps.tile([128, 2, BC], F32, tag="ps")
            nc.tensor.matmul(pe[:, 0, :], wek[:, g4, :], gg[:, g4, :],
                             start=True, stop=True)
            nc.tensor.matmul(pe[:, 1, :], wek[:, g4 + 1, :], gg[:, g4 + 1, :],
                             start=True, stop=True)
            copy(yy[:, g4:g4 + 2, :], pe[:])

        # ---- regroup2: yh[n2, c', i, b] = yy[COL2(c', i, f_loc), g4, b];
        # same stride-4 source spread as regroup1
        yh = yh_p.tile([128, 2, KOUT, BC], F16, tag="yh")
        for i in range(KOUT):
            for cp in range(2):
                base = 64 * cp + 16 * (i // 4) + (i % 4)
                ring().dma_start(
                    yh[:, cp, i, :],
                    yy[base:base + 13:4, :, :],
                )

        # ---- stage I: inverse DFT, operator stationary -> outT[m, b]
        for i2 in range(8):
            ot = os_p.tile([128, 4, BC], F16, tag="os")
            for il in range(2):
                i = 2 * i2 + il
                pi = ps.tile([128, 2, BC], F32, tag="ps")
                for mp in range(2):
                    nc.tensor.matmul(pi[:, mp, :], dmat[:, 0, mp, :],
                                     yh[:, 0, i, :], start=True, stop=False)
                    nc.tensor.matmul(pi[:, mp, :], dmat[:, 1, mp, :],
                                     yh[:, 1, i, :], start=False, stop=True)
                copy(ot[:, 2 * il:2 * il + 2, :], pi[:])
            nc.scalar.dma_start(out_d[:, 4 * i2:4 * (i2 + 1), :], ot[:])

    nc.compile()
    return nc


def _get_program():
    if "nc" not in _CACHE:
        _CACHE["nc"] = _build_program()
    return _CACHE["nc"]


def _install_ntff_hook():
    """Provide antenv.axon_hooks (absent in this image) so that
    run_bass_kernel_spmd(trace=True) can capture NTFF profiles through the
    axon client library."""
    import sys
    import types
    import ctypes
    import contextlib

    if "antenv.axon_hooks" in sys.modules:
        return
    try:
        lib = ctypes.CDLL("/opt/axon/libaxon_pjrt.so")
    except OSError:
        return
    if not hasattr(lib, "axon_start_nrt_profile"):
        return
    lib.axon_start_nrt_profile.argtypes = [
        ctypes.POINTER(ctypes.c_int64),
        ctypes.c_size_t,
    ]
    lib.axon_start_nrt_profile.restype = ctypes.c_int64
    lib.axon_stop_nrt_profile.argtypes = [ctypes.c_char_p]
    lib.axon_stop_nrt_profile.restype = ctypes.c_int64

    @contextlib.contextmanager
    def _hook(output_dir, device_ids):
        import jax

        jax.devices()
        if device_ids:
            ids = (ctypes.c_int64 * len(device_ids))(*device_ids)
            rc = lib.axon_start_nrt_profile(ids, len(device_ids))
        else:
            rc = lib.axon_start_nrt_profile(None, 0)
        if rc != 0:
            raise RuntimeError(f"axon_start_nrt_profile rc={rc}")
        try:
            yield
        finally:
            n = lib.axon_stop_nrt_profile(str(output_dir).encode())
            print(f"ntff profile: {n} file(s) -> {output_dir}")

    mod = types.ModuleType("antenv.axon_hooks")
    state = {"hook": _hook}
    mod.get_axon_ntff_profile_hook = lambda: state["hook"]
    mod.set_axon_ntff_profile_hook = lambda h: state.update(hook=h)
    sys.modules["antenv.axon_hooks"] = mod
    import antenv

    antenv.axon_hooks = mod


def kernel(x, W_real, W_imag, block_size, out_features):
    global LAST_RESULTS
    x = np.asarray(x, dtype=np.float32)
    Wr = np.asarray(W_real, dtype=np.float32)
    Wi = np.asarray(W_imag, dtype=np.float32)
    assert int(block_size) == BS and int(out_features) == D_OUT
    assert x.shape == (B_FULL, D_IN) and Wr.shape == (KOUT, KIN, 129)

    nc = _get_program()
    consts = _build_consts(Wr, Wi)
    # host-side shard + transpose + cast:
    # xt[c, t_lo, s, b] = x[c*512 + b, s*128 + t_lo]
    x16 = x.astype(np.float16)
    xt = np.ascontiguousarray(
        x16.reshape(NCORES, BC, 32, 128).transpose(0, 3, 2, 1)
    )
    core_ids = list(range(NCORES))
    in_maps = [{"xt": xt[c], **consts} for c in core_ids]
    trace = bool(int(os.environ.get("KERNEL_TRACE", "0")))
    if trace:
        _install_ntff_hook()
    res = run_bass_kernel_spmd(nc, in_maps, core_ids, trace=trace)
    LAST_RESULTS = res
    # out_c[m_lo, oc, b] -> out[c*512 + b, oc*128 + m_lo]
    outs = np.stack([res.results[c]["out"] for c in core_ids])
    out = outs.transpose(0, 3, 2, 1).reshape(B_FULL, D_OUT).astype(np.float32)
    return np.ascontiguousarray(out)


# revision 15
# speedup vs baseline: 1.0079x; 1.0079x over previous
"""Trainium2 Bass kernel for BlockFFTDirectPrior (v5).

Computes out = irfft(einsum('bjn,ijn->bin', rfft(x_blocks), conj(W)))
reshaped to [B, 4096], for x [4096, 4096] f32, W [16, 16, 129] complex
(block size 256).

Strategy: data-parallel over the batch axis across 8 NeuronCores (512 rows
each). The host pre-transposes each core's x slice to [d, b] layout and
casts to fp16 (no on-chip transpose stage, half the HBM traffic); the
kernel writes its output transposed [d, b] fp16 and the host transposes/
upcasts back.  All on-chip operands are fp16: the PE streams 2-byte moving
operands at 1 column/cycle (vs ~2 cycles for fp32/fp32r) and all
SBUF<->SBUF regroup DMAs move half the bytes.

Per core, three PE stages (no transposes):

  F: real DFT per input block j as matmuls (contract t in 2 chunks of 128)
       -> zf[n, c, j, b]; c=0 rows hold Re X[n] (n=0..127), c=1 rows hold
       Im X[n] for n>=1 and X[128] (real Nyquist bin) in the n=0 slot.
  E: per-frequency 16x16 complex mixing packed as real 2x2 blocks, four
     frequencies per 128x128 matmul -> one matmul per frequency group g4
     (32 total, no accumulation).  The Nyquist product is folded into the
     g4=0 operator (it lands in the otherwise-meaningless Im Y[0] slot).
  I: real inverse DFT with the operator stationary: out.T[m, b] chunks;
     only 4 distinct stationaries (dmat[c', mp]).

Between stages, two partition regroups (F->E "r1", E->I "r2") ride
round-robin on the three DMA rings (sync/scalar HWDGE + gpsimd SWDGE).
SBUF<->SBUF DMA costs read+write fabric-port bandwidth, so the batch is
processed as two pipelined halves of 256: each half's regroup burst
overlaps the other half's matmuls, keeping the PE HAM clock gate warm and
hiding most of the ~25 MB of DMA port traffic.  PSUM->SBUF copies pair
two matmul outputs per copy and alternate between DVE and ACT.
"""

import os
import numpy as np
from contextlib import ExitStack

import concourse.bass as bass
import concourse.tile as tile
from concourse import bacc, mybir
from concourse.bass_utils import run_bass_kernel_spmd

NCORES = 8
B_FULL, D_IN, D_OUT, BS = 4096, 4096, 4096, 256
BC = B_FULL // NCORES          # 512 batch rows per core
BH = BC // 2                   # 256 batch rows per pipelined half
KIN = KOUT = 16
F16 = mybir.dt.float16
F32 = mybir.dt.float32

_CACHE = {}
LAST_RESULTS = None            # BassKernelResults of the most recent run


def _build_consts(W_real, W_imag):
    """Constant matrices in the exact SBUF layouts the kernel reads.

    Frequency slot r in stage-F output order is frequency n = r, grouped
    for stage E as (g4 = n // 4, f_loc = n % 4).  Stage-I row order is
    n2 = f_loc * 32 + g4 (the order regroup-2's flat pairing produces).
    """
    f16 = np.float16
    t = np.arange(BS)
    n = np.arange(128)
    ang = 2.0 * np.pi / BS

    # zf physical row for frequency n (g4 = n//4, f = n%4) is
    # ROW1[n] = 16*(g4//4) + g4%4 + 4*f: each group's 4 rows sit at
    # partition stride 4 so every regroup-1 DMA's reads spread over 4
    # SDMA engines (descriptor->engine assignment keys on src partition).
    ROW1 = np.empty(128, dtype=np.int64)
    for nf_ in range(128):
        g4_, f_ = nf_ // 4, nf_ % 4
        ROW1[nf_] = 16 * (g4_ // 4) + (g4_ % 4) + 4 * f_

    # cfs[t_lo, tc, c, row]: column `row` holds frequency n with ROW1[n]=row
    cfs = np.zeros((128, 2, 2, 128), dtype=np.float32)
    C0 = np.cos(ang * np.outer(t, n))
    C1 = np.empty((BS, 128), dtype=np.float64)
    C1[:, 0] = np.cos(np.pi * t)                      # Nyquist row -> Xr[128]
    C1[:, 1:] = -np.sin(ang * np.outer(t, np.arange(1, 128)))
    inv1 = np.argsort(ROW1)                           # row -> frequency
    for tc in range(2):
        cfs[:, tc, 0, :] = C0[tc * 128:(tc + 1) * 128][:, inv1]
        cfs[:, tc, 1, :] = C1[tc * 128:(tc + 1) * 128][:, inv1]

    # yy physical row for (c', i, f): COL2 = c'*64 + 16*(i//4) + i%4 + 4*f
    # (same stride-4 trick for regroup-2's source reads)
    def COL2(cp_, i_, f_):
        return cp_ * 64 + 16 * (i_ // 4) + (i_ % 4) + 4 * f_

    # wek[row, g4, col]; row = f_loc*32 + c*16 + j, col = COL2(c', i, f_loc)
    wek = np.zeros((128, 32, 128), dtype=np.float32)
    fl = np.arange(4)[:, None, None]                  # f_loc
    ii = np.arange(KOUT)[None, :, None]
    jj = np.arange(KIN)[None, None, :]
    for g4 in range(32):
        nf = 4 * g4 + fl                              # frequency
        Wr = W_real[ii, jj, nf]
        Wi = W_imag[ii, jj, nf]
        # Yr = Wr*Xr + Wi*Xi ; Yi = Wr*Xi - Wi*Xr
        col0 = 0 * 64 + 16 * (ii // 4) + (ii % 4) + 4 * fl
        col1 = 64 + 16 * (ii // 4) + (ii % 4) + 4 * fl
        wek[fl * 32 + 0 * 16 + jj, g4, col0] = Wr
        wek[fl * 32 + 1 * 16 + jj, g4, col0] = Wi
        wek[fl * 32 + 1 * 16 + jj, g4, col1] = Wr
        wek[fl * 32 + 0 * 16 + jj, g4, col1] = -Wi
    # Nyquist fold: Im Y[0] slot carries Yr[128] = Wr[:, :, 128] * Xr[128]
    for i in range(KOUT):
        for j in range(KIN):
            wek[0 * 32 + 1 * 16 + j, 0, COL2(1, i, 0)] = W_real[i, j, 128]
            wek[0 * 32 + 0 * 16 + j, 0, COL2(1, i, 0)] = 0.0

    # dmat[n2, c', mp, m_lo]
    m = np.arange(BS)
    dmat = np.zeros((128, 2, 2, 128), dtype=np.float32)
    for nf in range(128):
        f_loc, g4 = nf % 4, nf // 4
        n2 = f_loc * 32 + g4
        if nf == 0:
            d0 = np.full(BS, 1.0 / BS)
            d1 = ((-1.0) ** m) / BS                   # Nyquist slot
        else:
            d0 = (2.0 / BS) * np.cos(ang * nf * m)
            d1 = -(2.0 / BS) * np.sin(ang * nf * m)
        for mp in range(2):
            dmat[n2, 0, mp] = d0[mp * 128:(mp + 1) * 128]
            dmat[n2, 1, mp] = d1[mp * 128:(mp + 1) * 128]

    return {
        "cfs": cfs.astype(f16),
        "wek": wek.astype(f16),
        "dmat": dmat.astype(f16),
    }


def _build_program():
    nc = bacc.Bacc(
        "TRN2", target_bir_lowering=False, debug=False, num_devices=NCORES
    )
    # xt[t_lo, s, b]: host-pretransposed fp16 x slice; d = s*128 + t_lo
    xt_d = nc.dram_tensor("xt", [128, 32, BC], F16, kind="ExternalInput").ap()
    cfs_d = nc.dram_tensor("cfs", [128, 2, 2, 128], F16, kind="ExternalInput").ap()
    wek_d = nc.dram_tensor("wek", [128, 32, 128], F16, kind="ExternalInput").ap()
    dmat_d = nc.dram_tensor("dmat", [128, 2, 2, 128], F16, kind="ExternalInput").ap()
    # out[m_lo, oc, b]: transposed fp16 output; d = oc*128 + m_lo, oc = i*2+mp
    out_d = nc.dram_tensor("out", [128, 32, BC], F16, kind="ExternalOutput").ap()

    cp_state = [0]
    ring_state = [0]

    with tile.TileContext(nc) as tc, ExitStack() as ctx:
        def copy(dst, src):
            # alternate PSUM->SBUF copies between DVE and ACT (ACT issues no
            # DMAs in this kernel, so copies never queue behind dispatches)
            if cp_state[0] % 2 == 0:
                nc.vector.tensor_copy(dst, src)
            else:
                nc.scalar.copy(dst, src)
            cp_state[0] += 1

        def ring():
            # weighted split: SWDGE (gpsimd) spreads descriptors evenly over
            # all 16 SDMA engines but dispatches serially (~0.8us); HWDGE
            # (sync) dispatches fast but skews onto 4 hot engines for these
            # few-source-partition patterns.  2:1 gpsimd:sync balances them.
            r = (nc.gpsimd, nc.gpsimd, nc.sync)[ring_state[0] % 3]
            ring_state[0] += 1
            return r

        consts = ctx.enter_context(tc.tile_pool(name="consts", bufs=1))
        xin_p = ctx.enter_context(tc.tile_pool(name="xin", bufs=1))
        zf_p = ctx.enter_context(tc.tile_pool(name="zf", bufs=1))
        yy_p = ctx.enter_context(tc.tile_pool(name="yy", bufs=1))
        yh_p = ctx.enter_context(tc.tile_pool(name="yh", bufs=1))
        gg_p = ctx.enter_context(tc.tile_pool(name="gg", bufs=1))
        os_p = ctx.enter_context(tc.tile_pool(name="os", bufs=3))
        ps = ctx.enter_context(tc.tile_pool(name="ps", bufs=4, space="PSUM"))

        cfs = consts.tile([128, 2, 2, 128], F16)
        wek = consts.tile([128, 32, 128], F16)
        dmat = consts.tile([128, 2, 2, 128], F16)
        wrm = consts.tile([128, 128], F16)

        xin = xin_p.tile([128, 32, BC], F16, tag="xin")
        # x chunks interleave sync/scalar (plain 128-partition patterns
        # spread fine on HWDGE) in stage-F consumption order, 512KB each;
        # gpsimd stays empty so regroup-1 dispatches the moment zf is ready
        nc.scalar.dma_start(cfs[:], cfs_d)
        for k in range(4):
            nc.sync.dma_start(xin[:, 8 * k:8 * k + 4, :],
                              xt_d[:, 8 * k:8 * k + 4, :])
            nc.scalar.dma_start(xin[:, 8 * k + 4:8 * k + 8, :],
                                xt_d[:, 8 * k + 4:8 * k + 8, :])
        nc.scalar.dma_start(wek[:], wek_d)
        nc.scalar.dma_start(dmat[:], dmat_d)

        # ---- PE warm-up: back-to-back matmuls on a memset tile (no DMA
        # dependency) hold the PE busy through the input-DMA latency so the
        # HAM clock gate is at 8/8 (2.4 GHz) when stage F starts.
        nc.vector.memset(wrm[:], 0)
        pw = ps.tile([128, 2, BC], F32, tag="ps")
        for _ in range(48):
            nc.tensor.matmul(pw[:, 0, 0:128], wrm[:], wrm[:],
                             start=True, stop=True)

        # ---- stage F: real DFT per block j (4 matmuls each, N=512)
        zf = zf_p.tile([128, 2, KIN, BC], F16, tag="zf")
        for j in range(KIN):
            pf = ps.tile([128, 2, BC], F32, tag="ps")
            for c in range(2):
                for tcx in range(2):
                    nc.tensor.matmul(
                        pf[:, c, :],
                        cfs[:, tcx, c, :],
                        xin[:, 2 * j + tcx, :],
                        start=(tcx == 0),
                        stop=(tcx == 1),
                    )
            copy(zf[:, :, j, :], pf[:])

        # ---- regroup1 + stage E, pipelined per group pair
        # gg[p, b] = zf[ROW1(4*g4 + f_loc), c, j, b], p = f_loc*32 + c*16 + j;
        # group g4's four zf rows sit at stride 4 (rows 16*(g4//4) + g4%4
        # + 4*f) so each DMA's source reads spread over 4 SDMA engines
        yy = yy_p.tile([128, 32, BC], F16, tag="yy")
        gg = gg_p.tile([128, 32, BC], F16, tag="gg")
        for g4 in range(32):
            base = 16 * (g4 // 4) + (g4 % 4)
            eng = nc.sync if g4 < 2 else ring()
            eng.dma_start(gg[:, g4, :], zf[base:base + 13:4, :, :, :])
        for g4 in range(0, 32, 2):
            pe = ps.tile([128, 2, BC], F32, tag="ps")
            nc.tensor.matmul(pe[:, 0, :], wek[:, g4, :], gg[:, g4, :],
                             start=True, stop=True)
            nc.tensor.matmul(pe[:, 1, :], wek[:, g4 + 1, :], gg[:, g4 + 1, :],
                             start=True, stop=True)
            copy(yy[:, g4:g4 + 2, :], pe[:])

        # ---- regroup2: yh[n2, c', i, b] = yy[COL2(c', i, f_loc), g4, b];
        # same stride-4 source spread as regroup1
        yh = yh_p.tile([128, 2, KOUT, BC], F16, tag="yh")
        for i in range(KOUT):
            for cp in range(2):
                base = 64 * cp + 16 * (i // 4) + (i % 4)
                eng = nc.sync if i == 0 else ring()
                eng.dma_start(
                    yh[:, cp, i, :],
                    yy[base:base + 13:4, :, :],
                )

        # ---- stage I: inverse DFT, operator stationary -> outT[m, b]
        for i2 in range(8):
            ot = os_p.tile([128, 4, BC], F16, tag="os")
            for il in range(2):
                i = 2 * i2 + il
                pi = ps.tile([128, 2, BC], F32, tag="ps")
                for mp in range(2):
                    nc.tensor.matmul(pi[:, mp, :], dmat[:, 0, mp, :],
                                     yh[:, 0, i, :], start=True, stop=False)
                    nc.tensor.matmul(pi[:, mp, :], dmat[:, 1, mp, :],
                                     yh[:, 1, i, :], start=False, stop=True)
                copy(ot[:, 2 * il:2 * il + 2, :], pi[:])
            nc.sync.dma_start(out_d[:, 4 * i2:4 * (i2 + 1), :], ot[:])

    nc.compile()
    return nc


def _get_program():
    if "nc" not in _CACHE:
        _CACHE["nc"] = _build_program()
    return _CACHE["nc"]


def _install_ntff_hook():
    """Provide antenv.axon_hooks (absent in this image) so that
    run_bass_kernel_spmd(trace=True) can capture NTFF profiles through the
    axon client library."""
    import sys
    import types
    import ctypes
    import contextlib

    if "antenv.axon_hooks" in sys.modules:
        return
    try:
        lib = ctypes.CDLL("/opt/axon/libaxon_pjrt.so")
    except OSError:
        return
    if not hasattr(lib, "axon_start_nrt_profile"):
        return
    lib.axon_start_nrt_profile.argtypes = [
        ctypes.POINTER(ctypes.c_int64),
        ctypes.c_size_t,
    ]
    lib.axon_start_nrt_profile.restype = ctypes.c_int64
    lib.axon_stop_nrt_profile.argtypes = [ctypes.c_char_p]
    lib.axon_stop_nrt_profile.restype = ctypes.c_int64

    @contextlib.contextmanager
    def _hook(output_dir, device_ids):
        import jax

        jax.devices()
        if device_ids:
            ids = (ctypes.c_int64 * len(device_ids))(*device_ids)
            rc = lib.axon_start_nrt_profile(ids, len(device_ids))
        else:
            rc = lib.axon_start_nrt_profile(None, 0)
        if rc != 0:
            raise RuntimeError(f"axon_start_nrt_profile rc={rc}")
        try:
            yield
        finally:
            n = lib.axon_stop_nrt_profile(str(output_dir).encode())
            print(f"ntff profile: {n} file(s) -> {output_dir}")

    mod = types.ModuleType("antenv.axon_hooks")
    state = {"hook": _hook}
    mod.get_axon_ntff_profile_hook = lambda: state["hook"]
    mod.set_axon_ntff_profile_hook = lambda h: state.update(hook=h)
    sys.modules["antenv.axon_hooks"] = mod
    import antenv

    antenv.axon_hooks = mod


def kernel(x, W_real, W_imag, block_size, out_features):
    global LAST_RESULTS
    x = np.asarray(x, dtype=np.float32)
    Wr = np.asarray(W_real, dtype=np.float32)
    Wi = np.asarray(W_imag, dtype=np.float32)
    assert int(block_size) == BS and int(out_features) == D_OUT
    assert x.shape == (B_FULL, D_IN) and Wr.shape == (KOUT, KIN, 129)

    nc = _get_program()
    consts = _build_consts(Wr, Wi)
    # host-side shard + transpose + cast:
    # xt[c, t_lo, s, b] = x[c*512 + b, s*128 + t_lo]
    x16 = x.astype(np.float16)
    xt = np.ascontiguousarray(
        x16.reshape(NCORES, BC, 32, 128).transpose(0, 3, 2, 1)
    )
    core_ids = list(range(NCORES))
    in_maps = [{"xt": xt[c], **consts} for c in core_ids]
    trace = bool(int(os.environ.get("KERNEL_TRACE", "0")))
    if trace:
        _install_ntff_hook()
    res = run_bass_kernel_spmd(nc, in_maps, core_ids, trace=trace)
    LAST_RESULTS = res
    # out_c[m_lo, oc, b] -> out[c*512 + b, oc*128 + m_lo]
    outs = np.stack([res.results[c]["out"] for c in core_ids])
    out = outs.transpose(0, 3, 2, 1).reshape(B_FULL, D_OUT).astype(np.float32)
    return np.ascontiguousarray(out)
